# revision 7
# baseline (speedup 1.0000x reference)
"""NoisyNet dense layer (training mode) on 8 TRN2 NeuronCores.

out[b,u] = x @ W_mu + eps_out * ((x*eps_in) @ W_sigma) + bias_mu + bias_sigma*eps_out

Sharding: data-parallel over batch (4096 -> 512 rows/core), weights/biases
replicated. On-device math runs in a transposed layout ([D,B]/[U,B]) so the
contraction dim D lands on SBUF partitions; the host does the (free)
transposes, dtype casts and the final gather.

v2 vs baseline:
 - Noise GEMM runs F8 of its 8 256-deep contraction chunks as fp8e4
   DoubleRow matmuls (2 MACs/cell/cycle); the rest stays bf16. F8 is
   chosen so the added quantization error keeps total rel err < 2e-2
   (fp8 full = 2.3e-2, F8=5 -> 1.85e-2, F8=4 -> 1.66e-2 on seed-0 data).
   Scale convention: z = x*(eps_in/4) [host prescales eps_in], W_sigma
   is scaled x2048 on host; PSUM holds 512*noise, epilogue ACT applies
   2^-9 * psum + bias_sigma in one op.
 - Input DMA is spread across the 3 DMA-capable queues (scalar/sync/
   gpsimd) so x lands in ~5us and the PE goes dense+warm early.
 - W_mu / W_sigma streams alternate sync(even u) / gpsimd(odd u) queues,
   halving per-queue bandwidth demand (baseline ran one queue at its
   saturation rate and stalled periodically).
 - Output tiles are written as bf16 (host casts to fp32), halving
   writeback bytes; out DMAs go on the scalar queue right after each
   epilogue so the writeback overlaps phase 2.
"""

import numpy as np
import ml_dtypes

import concourse.bacc as bacc
import concourse.mybir as mybir
import concourse.tile as tile
from concourse.bass_utils import run_bass_kernel_spmd

N_CORES = 8
B, D, U = 4096, 2048, 2048
BL = B // N_CORES          # 512 batch rows per core
P = 128                    # partitions
KT = D // P                # 16 contraction tiles of 128
UT = U // P                # 16 output tiles of 128
F8 = 5                     # DoubleRow fp8 chunks (256 contraction each)
KB0 = 2 * F8               # first bf16 k-tile of the noise GEMM
BF16 = mybir.dt.bfloat16
FP32 = mybir.dt.float32
FP8 = mybir.dt.float8e4
DR = mybir.MatmulPerfMode.DoubleRow
IDENT = mybir.ActivationFunctionType.Identity

_NBF = ml_dtypes.bfloat16
_NF8 = ml_dtypes.float8_e4m3   # IEEE-style e4m3, max +-240 == TRN FP8_EXP4

_cached = None


def _build():
    nc = bacc.Bacc("TRN2", target_bir_lowering=False, debug=False)

    # activations laid out [P, KT, BL]: partition p holds d = k*128+p
    xT = nc.declare_dram_parameter("xT", [P, KT, BL], BF16, isOutput=False)
    ei4T = nc.declare_dram_parameter("ei4T", [P, KT, BL], BF16, isOutput=False)
    eoT = nc.declare_dram_parameter("eoT", [P, UT, BL], BF16, isOutput=False)
    wmu = nc.declare_dram_parameter("wmu", [UT, P, KT * P], BF16, isOutput=False)
    # W_sigma*2048: fp8 DoubleRow part [u][p, kt, i, m] (d = kt*256+i*128+p)
    ws8 = nc.declare_dram_parameter("ws8", [UT, P, F8, 2, P], FP8, isOutput=False)
    # ... and bf16 tail part for k-tiles KB0..15
    wsb = nc.declare_dram_parameter("wsb", [UT, P, (KT - KB0) * P], BF16,
                                    isOutput=False)
    bmu = nc.declare_dram_parameter("bmu", [P, UT], FP32, isOutput=False)
    bsg = nc.declare_dram_parameter("bsg", [P, UT], FP32, isOutput=False)
    outT = nc.declare_dram_parameter("outT", [UT, P, BL], BF16, isOutput=True)

    with tile.TileContext(nc) as tc:
        with (
            tc.tile_pool(name="acts", bufs=1) as acts,
            tc.tile_pool(name="wm", bufs=9) as wmp,
            tc.tile_pool(name="ws", bufs=32) as wsp,
            tc.tile_pool(name="bias", bufs=1) as bp,
            tc.tile_pool(name="psum", bufs=4, space="PSUM") as pp,
            tc.tile_pool(name="psumn", bufs=4, space="PSUM") as ppn,
            tc.tile_pool(name="mean", bufs=1) as mp,
            tc.tile_pool(name="tmp", bufs=2) as tp,
            tc.tile_pool(name="out", bufs=3) as op,
        ):
            # HAM warm-up: matmuls on zeroed SBUF during the initial DMA wait
            # so the first real matmuls run closer to 2.4 GHz.
            warm_in = bp.tile([P, BL], BF16, tag="warmin")
            nc.gpsimd.memset(warm_in[:], 0.0)
            warm_ps = ppn.tile([P, BL], FP32, tag="psn")
            for _ in range(6):
                nc.tensor.matmul(warm_ps[:], warm_in[:, :P], warm_in[:])

            # ---- input streams -------------------------------------------
            x_sb = acts.tile([P, KT, BL], BF16, tag="x")
            ei_sb = acts.tile([P, KT, BL], BF16, tag="ei")
            eo_sb = acts.tile([P, UT, BL], BF16, tag="eo")
            z8_sb = acts.tile([P, F8, 2, BL], FP8, tag="z8")

            wm_tiles = {}
            ws_tiles = {}

            def xch(c, q):
                s = slice(2 * c, 2 * c + 2)
                q.dma_start(x_sb[:, s, :], xT[:, s, :])

            def fetch_wm(u, q):
                wm = wmp.tile([P, KT * P], BF16, tag="wm")
                q.dma_start(wm[:], wmu[u])
                wm_tiles[u] = wm

            def fetch_ws(u, q):
                w8 = wsp.tile([P, F8, 2, P], FP8, tag="w8")
                q.dma_start(w8[:], ws8[u])
                wb = wsp.tile([P, (KT - KB0) * P], BF16, tag="wb")
                q.dma_start(wb[:], wsb[u])
                ws_tiles[u] = (w8, wb)

            # Per-queue issue order is transfer order; every queue runs at
            # ~115 GB/s, so spread the load and put early-deadline data first.
            # sync: wm0(head) x1 wm0(rest) x5 wm2 | wm4 wm6 wm8 wm10 wm12 ws0 ws4 ws8 ws12
            # gpsimd: x3 wm1 x6 bias x7 wm3 | wm5 wm7 wm9 wm11 ws1 ws3 ... ws15
            # scalar: x0 x2 x4 ei*4 | wm13 wm14 wm15 eo*4 ws2 ws6 ws10 ws14 | outs
            wm0 = wmp.tile([P, KT * P], BF16, tag="wm")
            nc.sync.dma_start(wm0[:, :4 * P], wmu[0][:, :4 * P])
            wm_tiles[0] = wm0
            xch(0, nc.scalar)
            xch(1, nc.sync)
            xch(3, nc.gpsimd)
            nc.sync.dma_start(wm0[:, 4 * P:], wmu[0][:, 4 * P:])
            xch(2, nc.scalar)
            fetch_wm(1, nc.gpsimd)
            xch(5, nc.sync)
            xch(6, nc.gpsimd)
            xch(4, nc.scalar)
            fetch_wm(2, nc.sync)
            bmu_t = bp.tile([P, UT], FP32, tag="bmu")
            nc.gpsimd.dma_start(bmu_t[:], bmu[:])
            bsg_t = bp.tile([P, UT], FP32, tag="bsg")
            nc.gpsimd.dma_start(bsg_t[:], bsg[:])
            xch(7, nc.gpsimd)
            fetch_wm(3, nc.gpsimd)

            # eps_in on the scalar queue (needed mid phase 1 for z tiles)
            for c in range(4):
                s = slice(4 * c, 4 * c + 4)
                nc.scalar.dma_start(ei_sb[:, s, :], ei4T[:, s, :])

            # z tiles on DVE as soon as x+eps_in chunks land:
            # z8[:,kt,:,:] (fp8) covers k-tiles 2kt..2kt+1; zb bf16 tail.
            for kt in range(F8):
                s = slice(2 * kt, 2 * kt + 2)
                nc.vector.tensor_mul(z8_sb[:, kt, :, :], x_sb[:, s, :],
                                     ei_sb[:, s, :])
            # bf16 z tail computed in place into ei_sb (frees 6KB/partition)
            for j0 in range(KB0, KT, 2):
                s = slice(j0, min(j0 + 2, KT))
                nc.vector.tensor_mul(ei_sb[:, s, :],
                                     x_sb[:, s, :], ei_sb[:, s, :])

            # remaining fetches, keyed by phase-1 iteration at which they are
            # emitted into their engine's program (transfers start earlier if
            # the queue is free; emission points only bound the issue order).
            sched = {
                0: [("wm", 4, "sync")],
                1: [("wm", 5, "gpsimd")],
                2: [("wm", 6, "sync")],
                3: [("wm", 7, "gpsimd"), ("wm", 13, "scalar")],
                4: [("wm", 8, "sync"), ("wm", 14, "scalar")],
                5: [("wm", 9, "gpsimd"), ("wm", 15, "scalar")],
                6: [("wm", 10, "sync"), ("eo", 0, "scalar")],
                7: [("wm", 11, "gpsimd"), ("eo", 1, "scalar")],
                8: [("wm", 12, "sync"), ("eo", 2, "scalar")],
                9: [("ws", 1, "gpsimd"), ("eo", 3, "scalar")],
                10: [("ws", 0, "sync"), ("ws", 3, "gpsimd")],
                11: [("ws", 5, "gpsimd"), ("ws", 2, "scalar")],
                12: [("ws", 4, "sync"), ("ws", 7, "gpsimd")],
                13: [("ws", 9, "gpsimd"), ("ws", 6, "scalar")],
                14: [("ws", 8, "sync"), ("ws", 11, "gpsimd")],
                15: [("ws", 13, "gpsimd"), ("ws", 10, "scalar")],
                16: [("ws", 12, "sync"), ("ws", 15, "gpsimd"),
                     ("ws", 14, "scalar")],
            }

            def emit(it):
                for kind, u, qn in sched.get(it, []):
                    q = getattr(nc, qn)
                    if kind == "wm":
                        fetch_wm(u, q)
                    elif kind == "ws":
                        fetch_ws(u, q)
                    else:
                        s = slice(4 * u, 4 * u + 4)
                        q.dma_start(eo_sb[:, s, :], eoT[:, s, :])

            # ---- Phase 1: mean terms. t_m[u] = W_mu[u].T @ x + bias_mu[u] ----
            t_m = []
            for u in range(UT):
                emit(u)
                wm = wm_tiles.pop(u)
                pm = pp.tile([P, BL], FP32, tag="psm")
                for k in range(KT):
                    nc.tensor.matmul(
                        pm[:], wm[:, k * P:(k + 1) * P], x_sb[:, k, :],
                        start=(k == 0), stop=(k == KT - 1),
                    )
                tm = mp.tile([P, BL], BF16, tag=f"tm{u}")
                nc.scalar.activation(tm[:], pm[:], IDENT,
                                     bias=bmu_t[:, u:u + 1], scale=1.0)
                t_m.append(tm)

            emit(16)

            # ---- Phase 2: noise terms + combine ----
            for u in range(UT):
                w8, wb = ws_tiles.pop(u)
                last = (u == UT - 1)
                halves = (0, BL // 2, BL) if last else (0, BL)
                for h in range(len(halves) - 1):
                    lo, hi = halves[h], halves[h + 1]
                    pn = ppn.tile([P, hi - lo], FP32, tag="psn")
                    for kt in range(F8):
                        nc.tensor.matmul(
                            pn[:], w8[:, kt, :, :], z8_sb[:, kt, :, lo:hi],
                            start=(kt == 0), stop=False, perf_mode=DR,
                        )
                    for j in range(KT - KB0):
                        nc.tensor.matmul(
                            pn[:], wb[:, j * P:(j + 1) * P],
                            ei_sb[:, KB0 + j, lo:hi],
                            start=(F8 == 0 and j == 0), stop=(j == KT - KB0 - 1),
                        )
                    t_n = tp.tile([P, hi - lo], BF16, tag="tn")
                    nc.scalar.activation(t_n[:], pn[:], IDENT,
                                         bias=bsg_t[:, u:u + 1], scale=2.0 ** -9)
                    pr = tp.tile([P, hi - lo], BF16, tag="pr")
                    nc.vector.tensor_mul(pr[:], t_n[:], eo_sb[:, u, lo:hi])
                    o = op.tile([P, hi - lo], BF16, tag="o")
                    nc.vector.tensor_add(o[:], pr[:], t_m[u][:, lo:hi])
                    nc.scalar.dma_start(outT[u][:, lo:hi], o[:])

    nc.compile()
    return nc


def _get_nc():
    global _cached
    if _cached is None:
        _cached = _build()
    return _cached


def host_prep(x, weight_mu, weight_sigma, bias_mu, bias_sigma, eps_in, eps_out):
    """Layout prep only: transposes, dtype casts/quantization, sharding."""
    def to_pkb(a):  # [B, D] -> per-core [P, KT, BL] (partition p holds k*128+p)
        a = np.ascontiguousarray(a.astype(_NBF))
        return [
            np.ascontiguousarray(
                a[c * BL:(c + 1) * BL].T.reshape(KT, P, BL).transpose(1, 0, 2))
            for c in range(N_CORES)
        ]

    xs = to_pkb(x)
    eis = to_pkb(eps_in * 0.25)
    eos = to_pkb(eps_out)  # same transform, u in place of k

    def w_blocks(w):  # [D(sub), U] -> [UT, P, kt*P] bf16
        kt = w.shape[0] // P
        wb = w.astype(_NBF).reshape(kt, P, UT, P).transpose(2, 1, 0, 3)
        return np.ascontiguousarray(wb.reshape(UT, P, kt * P))

    wmu_h = w_blocks(weight_mu)
    ws = weight_sigma * 2048.0
    # fp8 DoubleRow part: d = kt*256 + i*128 + p for k-tiles < KB0
    w8 = ws[:KB0 * P].reshape(F8, 2, P, UT, P).transpose(3, 2, 0, 1, 4)
    w8 = np.clip(w8, -240.0, 240.0).astype(_NF8)
    wsg8_h = np.ascontiguousarray(w8)
    wsb_h = w_blocks(ws[KB0 * P:])
    bmu_h = np.ascontiguousarray(bias_mu.astype(np.float32).reshape(UT, P).T)
    bsg_h = np.ascontiguousarray(bias_sigma.astype(np.float32).reshape(UT, P).T)

    return [
        {
            "xT": xs[c],
            "ei4T": eis[c],
            "eoT": eos[c],
            "wmu": wmu_h,
            "ws8": wsg8_h,
            "wsb": wsb_h,
            "bmu": bmu_h,
            "bsg": bsg_h,
        }
        for c in range(N_CORES)
    ]


def kernel(x, weight_mu, weight_sigma, bias_mu, bias_sigma, eps_in, eps_out,
           _trace=False):
    nc = _get_nc()
    in_maps = host_prep(x, weight_mu, weight_sigma, bias_mu, bias_sigma,
                        eps_in, eps_out)

    res = run_bass_kernel_spmd(nc, in_maps, core_ids=list(range(N_CORES)),
                               trace=_trace)
    kernel.last_result = res

    out = np.empty((B, U), dtype=np.float32)
    for c in range(N_CORES):
        oc = np.asarray(res.results[c]["outT"])  # [UT, P, BL] bf16
        out[c * BL:(c + 1) * BL] = (
            oc.transpose(2, 0, 1).reshape(BL, U).astype(np.float32))
    return out
